# revision 1
# baseline (speedup 1.0000x reference)
"""GridMask apply (BatchHide): out = feature * mask, mask broadcast over channels.

feature: [32, 128, 224, 224] f32, mask: [32, 1, 224, 224] f32.
Data-parallel over batch across 8 NeuronCores (4 samples per core).

Per-core layout: flatten H*W = 50176 = 128 * 392 and put the 128-chunk of
spatial positions on SBUF partitions, channels on the free dim. The mask tile
[128, 392] then has exactly the same partition mapping as every channel's
feature tile, so it is loaded once per sample and reused across all 128
channels via a free-dim (stride-0) broadcast AP — zero broadcast traffic.
"""

import numpy as np

import concourse.bacc as bacc
import concourse.tile as tile
from concourse import mybir
from concourse.bass_utils import run_bass_kernel_spmd

B, C, H, W = 32, 128, 224, 224
N_CORES = 8
B_LOC = B // N_CORES  # 4 samples per core
HW = H * W  # 50176
P = 128
F = HW // P  # 392
CK = 16  # channels per tile
F32 = mybir.dt.float32

_nc_cache = None


def _build():
    nc = bacc.Bacc("TRN2", target_bir_lowering=False, debug=False, num_devices=N_CORES)
    feat = nc.dram_tensor("feature", [B_LOC, C, HW], F32, kind="ExternalInput").ap()
    msk = nc.dram_tensor("mask", [B_LOC, HW], F32, kind="ExternalInput").ap()
    out = nc.dram_tensor("out", [B_LOC, C, HW], F32, kind="ExternalOutput").ap()

    with tile.TileContext(nc) as tc:
        with (
            tc.tile_pool(name="mask", bufs=2) as mpool,
            tc.tile_pool(name="data", bufs=5) as dpool,
        ):
            for b in range(B_LOC):
                mt = mpool.tile([P, F], F32)
                nc.sync.dma_start(out=mt[:], in_=msk[b].rearrange("(p f) -> p f", p=P))
                fv = feat[b].rearrange("c (p f) -> p c f", p=P)
                ov = out[b].rearrange("c (p f) -> p c f", p=P)
                for c0 in range(0, C, CK):
                    ft = dpool.tile([P, CK, F], F32)
                    nc.sync.dma_start(out=ft[:], in_=fv[:, c0 : c0 + CK, :])
                    nc.vector.tensor_mul(
                        out=ft[:],
                        in0=ft[:],
                        in1=mt[:, None, :].broadcast_to([P, CK, F]),
                    )
                    nc.scalar.dma_start(out=ov[:, c0 : c0 + CK, :], in_=ft[:])
    nc.compile()
    return nc


def _get_nc():
    global _nc_cache
    if _nc_cache is None:
        _nc_cache = _build()
    return _nc_cache


def kernel(feature, mask):
    feature = np.ascontiguousarray(np.asarray(feature, dtype=np.float32))
    mask = np.ascontiguousarray(np.asarray(mask, dtype=np.float32))
    nc = _get_nc()
    in_maps = [
        {
            "feature": feature[i * B_LOC : (i + 1) * B_LOC].reshape(B_LOC, C, HW),
            "mask": mask[i * B_LOC : (i + 1) * B_LOC].reshape(B_LOC, HW),
        }
        for i in range(N_CORES)
    ]
    res = run_bass_kernel_spmd(nc, in_maps, list(range(N_CORES))).results
    return np.concatenate(
        [res[i]["out"].reshape(B_LOC, C, H, W) for i in range(N_CORES)], axis=0
    )
